# revision 2
# baseline (speedup 1.0000x reference)
"""DeepSeekV3 router (moe_routing) Bass kernel for 8x TRN2 NeuronCores.

Data-parallel over tokens (T sharded 8 ways); kernel_DE/bias_E replicated.

z = x@W via fp32r lead plus two fp8e4m3 DoubleRow corrections:
  P   = (x_r * 2^12) @ W_r                 fp32r, N=256, 1 cyc/col
  C   = e4m3(x_e*2^12) @ e4m3(W*2^6)       DoubleRow, 0.5 cyc/col, K=256
      + e4m3(x_r*2)    @ e4m3(W_e*2^17)    DoubleRow (same 2^18 scale)
  z   = 2^-12 * P + 2^-18 * C
x_r = fp32r(x) (12-bit), x_e = x - x_r exactly.  The DR terms correct
x_e@W and x_r@W_e to ~2^-4 relative, so the total z error is ~2^-17 ->
a handful of top-k flips (sim: 6 of 131072).
PE cost/chunk: 256 transpose + 256 lead + 128 corrections = 640 cyc
vs 1024 for the 3-pass fp32r scheme.  C accumulates in its own PSUM bank
(interleaving two accumulation groups in one bank corrupts results).
"""

import numpy as np

import concourse.bass as bass
import concourse.mybir as mybir
from concourse import bacc
from concourse.bass_utils import run_bass_kernel_spmd
from concourse.masks import make_identity
from concourse.tile import TileContext

F32 = mybir.dt.float32
F32R = mybir.dt.float32r
FP8 = mybir.dt.float8e4
I32 = mybir.dt.int32
U32 = mybir.dt.uint32
DR = mybir.MatmulPerfMode.DoubleRow

T, D, E = 16384, 7168, 256
N_CORES = 8
TOP_K = 8
N_GROUPS = 8
TOPK_GROUPS = 4
EPG = E // N_GROUPS
SCALE = 2.5

P = 128
TS = T // N_CORES
KC = D // P                # 56 contraction chunks
TG = 8                     # chunks per group (PSUM stage + xt granularity)
NG = KC // TG              # 7 groups per tile
MM_LAG = 2                 # matmul groups lag transposes by this many steps

SX = 2.0**12               # x lead scale (P = 2^12 * xr@Wr)
SW8 = 2.0**6               # W fp8 scale
SWE = 2.0**17              # W_e fp8 scale; x_r8 = e4m3(x_r*2) so both
# correction products live at scale 2^18.  z*2^12 = P + C * 2^-6


def build(ts: int = TS) -> bass.Bass:
    nt = ts // P
    nc = bacc.Bacc("TRN2", target_bir_lowering=False)

    x_dram = nc.dram_tensor("x", [ts, D], F32, kind="ExternalInput")
    wr_dram = nc.dram_tensor("wr", [D, E], F32R, kind="ExternalInput")
    w8_dram = nc.dram_tensor("w8", [D, E], FP8, kind="ExternalInput")
    we8_dram = nc.dram_tensor("we8", [D, E], FP8, kind="ExternalInput")
    b_dram = nc.dram_tensor("bias", [E], F32, kind="ExternalInput")
    ow_dram = nc.dram_tensor("out_w", [ts, TOP_K], F32, kind="ExternalOutput")
    oi_dram = nc.dram_tensor("out_i", [ts, TOP_K], I32, kind="ExternalOutput")

    with TileContext(nc) as tc:
        with (
            tc.tile_pool(name="consts", bufs=1) as cp,
            tc.tile_pool(name="natp", bufs=9) as natp,
            tc.tile_pool(name="xtp", bufs=3) as xtp,
            tc.tile_pool(name="stg", bufs=2, space=bass.MemorySpace.PSUM) as stgp,
            tc.tile_pool(name="zp", bufs=2, space=bass.MemorySpace.PSUM) as zpp,
            tc.tile_pool(name="cr", bufs=2, space=bass.MemorySpace.PSUM) as crp,
            tc.tile_pool(name="sc", bufs=2) as scp,
            tc.tile_pool(name="rt", bufs=2) as rp,
            tc.tile_pool(name="outp", bufs=3) as op_,
        ):
            # ---- constants ----
            ident = cp.tile([P, P], F32)
            make_identity(nc, ident)

            bias_rep = cp.tile([P, E], F32)
            nc.gpsimd.dma_start(
                out=bias_rep,
                in_=bass.AP(tensor=b_dram, offset=0, ap=[[0, P], [1, E]]),
            )

            iota_i = cp.tile([P, E], I32)
            nc.gpsimd.iota(iota_i, pattern=[[1, E]], base=0, channel_multiplier=0)
            iota_f = cp.tile([P, E], F32)
            nc.vector.tensor_copy(iota_f, iota_i)

            # x eighth tiles, aligned 1:1 with transpose groups
            nat_tiles: dict[tuple, object] = {}

            def load_eighth(i, g):
                natq = natp.tile([P, TG * P], F32, tag="natq", name="natq")
                nat_tiles[(i, g)] = natq
                nc.sync.dma_start(
                    out=natq,
                    in_=x_dram[i * P : (i + 1) * P, g * TG * P : (g + 1) * TG * P],
                )

            # ---- resident weights, precomputed on host:
            # wr = 12-bit-rounded W (read exactly by the fp32r PE),
            # w8 = e4m3(W*2^6), we8 = e4m3((W - wr)*2^17) ----
            w_cat = cp.tile([P, KC, E], F32R)
            w8_cat = cp.tile([P, KC, E], FP8)
            we8_cat = cp.tile([P, KC, E], FP8)
            wr_re = wr_dram.rearrange("(c p) e -> p c e", p=P)
            w8_re = w8_dram.rearrange("(c p) e -> p c e", p=P)
            we8_re = we8_dram.rearrange("(c p) e -> p c e", p=P)

            def load_w_piece(tensor_cat, re_view, lo, hi):
                sl = slice(lo, hi)
                nc.gpsimd.dma_start(out=tensor_cat[:, sl, :], in_=re_view[:, sl, :])

            # W pieces go on the gpsimd queue (never the scalar queue: slow
            # strided triggers would head-of-line-block the xtrs ACT stream).
            # First wr piece is issued before x so step-2 matmuls have it.
            load_w_piece(w_cat, wr_re, 0, 8)
            load_eighth(0, 0)
            load_eighth(0, 1)
            load_w_piece(w8_cat, w8_re, 0, 8)
            load_w_piece(we8_cat, we8_re, 0, 8)
            load_eighth(0, 2)
            load_w_piece(w_cat, wr_re, 8, 24)
            load_eighth(0, 3)
            load_w_piece(w8_cat, w8_re, 8, 24)
            load_w_piece(we8_cat, we8_re, 8, 24)
            load_eighth(0, 4)
            load_eighth(0, 5)
            load_eighth(0, 6)

            deferred_w = [
                lambda: load_w_piece(w_cat, wr_re, 24, 40),
                lambda: load_w_piece(w8_cat, w8_re, 24, 40),
                lambda: load_w_piece(we8_cat, we8_re, 24, 40),
                lambda: load_w_piece(w_cat, wr_re, 40, 56),
                lambda: load_w_piece(w8_cat, w8_re, 40, 56),
                lambda: load_w_piece(we8_cat, we8_re, 40, 56),
            ]

            xt_tiles: dict[tuple, object] = {}
            z_tiles: dict[int, object] = {}
            c_tiles: dict[int, object] = {}

            def transpose_group(i, g):
                stage = stgp.tile([P, TG * P], F32, tag="stage")
                natq = nat_tiles[(i, g)]
                for j in range(TG):
                    nc.tensor.transpose(
                        stage[:, j * P : (j + 1) * P],
                        natq[:, j * P : (j + 1) * P],
                        ident,
                    )
                # xtrs = fp32r(x^T * 2^12): rounds AND evicts PSUM
                xtrs = xtp.tile([P, TG * P], F32R, tag="xtrs", name="xtrs")
                nc.scalar.activation(
                    xtrs, stage, mybir.ActivationFunctionType.Copy, scale=SX
                )
                # xte8 = e4m3(x_e * 2^12) = e4m3(stage*2^12 - xtrs)
                xte8 = xtp.tile([P, TG * P], FP8, tag="xte8", name="xte8")
                nc.vector.scalar_tensor_tensor(
                    xte8,
                    stage,
                    SX,
                    xtrs,
                    op0=mybir.AluOpType.mult,
                    op1=mybir.AluOpType.subtract,
                )
                # xtr8 = e4m3(x_r * 2) = e4m3(xtrs * 2^-11)
                xtr8 = xtp.tile([P, TG * P], FP8, tag="xtr8", name="xtr8")
                nc.scalar.activation(
                    xtr8, xtrs, mybir.ActivationFunctionType.Copy, scale=2.0**-11
                )
                xt_tiles[(i, g)] = (xtrs, xte8, xtr8)
                nat_tiles.pop((i, g))

            def matmul_group(i, g):
                xtrs, xte8, xtr8 = xt_tiles.pop((i, g))
                if i not in z_tiles:
                    z_tiles[i] = zpp.tile([P, E], F32, tag="z", name="z")
                    c_tiles[i] = crp.tile([P, E], F32, tag="c", name="c")
                z = z_tiles[i]
                corr = c_tiles[i]
                xtrs3 = xtrs.rearrange("p (c q) -> p c q", c=TG)
                xte83 = xte8.rearrange("p (c q) -> p c q", c=TG)
                xtr83 = xtr8.rearrange("p (c q) -> p c q", c=TG)
                for j in range(TG):
                    c = g * TG + j
                    nc.tensor.matmul(
                        z,
                        xtrs3[:, j, :],
                        w_cat[:, c, :],
                        start=(c == 0),
                        stop=(c == KC - 1),
                    )
                for j in range(0, TG, 2):
                    c = g * TG + j
                    nc.tensor.matmul(
                        corr,
                        xte83[:, j : j + 2, :],
                        w8_cat[:, c : c + 2, :],
                        start=(c == 0),
                        stop=False,
                        perf_mode=DR,
                    )
                    nc.tensor.matmul(
                        corr,
                        xtr83[:, j : j + 2, :],
                        we8_cat[:, c : c + 2, :],
                        start=False,
                        stop=(c == KC - 2),
                        perf_mode=DR,
                    )

            def routing_stages(i):
                st = {}

                def s0():
                    z = z_tiles.pop(i)
                    corr = c_tiles.pop(i)
                    # zt = 2^12 * z_total = P + C*2^-6
                    cc = scp.tile([P, E], F32, tag="cc", name="cc")
                    nc.scalar.activation(
                        cc, corr, mybir.ActivationFunctionType.Copy, scale=2.0**-6
                    )
                    zt = scp.tile([P, E], F32, tag="zt", name="zt")
                    nc.vector.scalar_tensor_tensor(
                        zt,
                        z,
                        1.0,
                        cc,
                        op0=mybir.AluOpType.mult,
                        op1=mybir.AluOpType.add,
                    )
                    st["zt"] = zt

                def s1():
                    scores = scp.tile([P, E], F32, tag="scores", name="scores")
                    nc.scalar.activation(
                        scores,
                        st["zt"],
                        mybir.ActivationFunctionType.Sigmoid,
                        scale=1.0 / SX,
                    )
                    biased = rp.tile([P, E], F32, tag="biased", name="biased")
                    nc.gpsimd.tensor_add(biased, scores, bias_rep)
                    st["scores"], st["biased"] = scores, biased

                def s2():
                    gmax = rp.tile([P, N_GROUPS * 8], F32, tag="gmax", name="gmax")
                    for g in range(N_GROUPS):
                        nc.vector.max(
                            gmax[:, g * 8 : (g + 1) * 8],
                            st["biased"][:, g * EPG : (g + 1) * EPG],
                        )
                    gm3 = gmax.rearrange("p (g k) -> p g k", k=8)
                    gsc = rp.tile([P, N_GROUPS], F32, tag="gsc", name="gsc")
                    gsc3 = gsc.rearrange("p (g k) -> p g k", k=1)
                    nc.vector.tensor_add(gsc3, gm3[:, :, 0:1], gm3[:, :, 1:2])
                    st["gsc"] = gsc

                def s3():
                    gsc = st["gsc"]
                    g8 = rp.tile([P, 8], F32, tag="g8", name="g8")
                    nc.vector.max(g8, gsc)
                    maskg = rp.tile([P, N_GROUPS], F32, tag="maskg", name="maskg")
                    nc.vector.tensor_scalar(
                        maskg,
                        gsc,
                        g8[:, TOPK_GROUPS - 1 : TOPK_GROUPS],
                        None,
                        op0=mybir.AluOpType.is_ge,
                    )
                    masked = rp.tile([P, E], F32, tag="masked", name="masked")
                    mg3 = maskg.rearrange("p (g k) -> p g k", k=1)
                    nc.gpsimd.tensor_tensor(
                        masked.rearrange("p (g e) -> p g e", g=N_GROUPS),
                        st["biased"].rearrange("p (g e) -> p g e", g=N_GROUPS),
                        mg3.to_broadcast([P, N_GROUPS, EPG]),
                        op=mybir.AluOpType.mult,
                    )
                    st["masked"] = masked

                def s4():
                    masked = st["masked"]
                    top8 = rp.tile([P, 8], F32, tag="top8", name="top8")
                    nc.vector.max(top8, masked)
                    idx = rp.tile([P, 8], U32, tag="idx", name="idx")
                    nc.vector.max_index(idx, top8, masked)
                    idxf = rp.tile([P, 8], F32, tag="idxf", name="idxf")
                    nc.vector.tensor_copy(idxf, idx)
                    st["idx"], st["idxf"] = idx, idxf

                def gather(wg, k0):
                    for k in range(k0, k0 + 4):
                        nc.vector.scalar_tensor_tensor(
                            st["scr"][k % 4],
                            iota_f,
                            st["idxf"][:, k : k + 1],
                            st["scores"],
                            op0=mybir.AluOpType.is_equal,
                            op1=mybir.AluOpType.mult,
                            accum_out=wg[:, k - k0 : k - k0 + 1],
                        )

                def s5():
                    # two wg tiles -> two independent 4-op accumulation chains
                    st["wga"] = rp.tile([P, 4], F32, tag="wga", name="wga")
                    st["wgb"] = rp.tile([P, 4], F32, tag="wgb", name="wgb")
                    st["scr"] = [
                        rp.tile([P, E], F32, tag=f"scratch{j}", name=f"scratch{j}")
                        for j in range(4)
                    ]
                    gather(st["wga"], 0)

                def s6():
                    gather(st["wgb"], 4)
                    wga, wgb = st["wga"], st["wgb"]
                    sa = rp.tile([P, 1], F32, tag="sa", name="sa")
                    nc.vector.tensor_reduce(
                        sa, wga, axis=mybir.AxisListType.X, op=mybir.AluOpType.add
                    )
                    ssum = rp.tile([P, 1], F32, tag="ssum", name="ssum")
                    nc.vector.tensor_reduce(
                        ssum, wgb, axis=mybir.AxisListType.X, op=mybir.AluOpType.add
                    )
                    nc.vector.tensor_tensor(
                        ssum, ssum, sa, op=mybir.AluOpType.add
                    )
                    nc.vector.tensor_scalar_add(ssum, ssum, 1e-20)
                    rinv = rp.tile([P, 1], F32, tag="rinv", name="rinv")
                    nc.vector.reciprocal(rinv, ssum)
                    nc.vector.tensor_scalar_mul(rinv, rinv, SCALE)
                    wout = op_.tile([P, TOP_K], F32, tag="wout", name="wout")
                    nc.vector.tensor_tensor(
                        wout[:, 0:4],
                        wga,
                        rinv.to_broadcast([P, 4]),
                        op=mybir.AluOpType.mult,
                    )
                    nc.vector.tensor_tensor(
                        wout[:, 4:8],
                        wgb,
                        rinv.to_broadcast([P, 4]),
                        op=mybir.AluOpType.mult,
                    )
                    iout = op_.tile([P, TOP_K], I32, tag="iout", name="iout")
                    nc.gpsimd.tensor_copy(iout, st["idx"])
                    nc.scalar.dma_start(out=ow_dram[i * P : (i + 1) * P, :], in_=wout)
                    nc.scalar.dma_start(out=oi_dram[i * P : (i + 1) * P, :], in_=iout)

                return [s0, s1, s2, s3, s4, s5, s6]

            # flat (tile, group) step stream; matmuls lag transposes by
            # MM_LAG; routing is staggered one stage per step so its vector
            # burst never head-of-line-blocks the xte8/xtrs pipeline
            from collections import deque

            pending = deque()
            steps = [(i, g) for i in range(nt) for g in range(NG)]
            for s, (i, g) in enumerate(steps):
                if i + 1 < nt:
                    load_eighth(i + 1, g)
                if deferred_w:
                    deferred_w.pop(0)()
                transpose_group(i, g)
                if s >= MM_LAG:
                    mi, mg = steps[s - MM_LAG]
                    matmul_group(mi, mg)
                    if mg == NG - 1:
                        pending.extend(routing_stages(mi))
                if pending:
                    pending.popleft()()
            for s in range(len(steps) - MM_LAG, len(steps)):
                mi, mg = steps[s]
                matmul_group(mi, mg)
                if mg == NG - 1:
                    pending.extend(routing_stages(mi))
                if pending:
                    pending.popleft()()
            while pending:
                pending.popleft()()

    nc.compile()
    return nc


def host_weights(kernel_DE: np.ndarray):
    """wr = W rounded to a 12-bit significand (fp32r-exact), plus the fp8
    operands of the correction products."""
    import ml_dtypes

    w32 = np.ascontiguousarray(kernel_DE, dtype=np.float32)
    i = w32.view(np.uint32).copy()
    shift = np.uint32(12)                      # keep 11 explicit mantissa bits
    half = np.uint32(1 << 11)
    lsb = (i >> shift) & np.uint32(1)
    i = (i + half - np.uint32(1) + lsb) & ~np.uint32((1 << 12) - 1)
    wr = i.view(np.float32)
    w8 = (w32 * SW8).astype(ml_dtypes.float8_e4m3)
    we8 = ((w32 - wr) * SWE).astype(ml_dtypes.float8_e4m3)
    return wr, w8, we8


def make_in_maps(inputs: dict) -> list[dict]:
    x_TD = np.ascontiguousarray(inputs["x_TD"], dtype=np.float32)
    bias_E = np.ascontiguousarray(inputs["bias_E"], dtype=np.float32)
    wr, w8, we8 = host_weights(inputs["kernel_DE"])
    return [
        {
            "x": x_TD[c * TS : (c + 1) * TS],
            "wr": wr,
            "w8": w8,
            "we8": we8,
            "bias": bias_E,
        }
        for c in range(N_CORES)
    ]


def kernel(x_TD: np.ndarray, kernel_DE: np.ndarray, bias_E: np.ndarray):
    nc = build(TS)
    in_maps = make_in_maps(
        {"x_TD": x_TD, "kernel_DE": kernel_DE, "bias_E": bias_E}
    )
    res = run_bass_kernel_spmd(nc, in_maps, list(range(N_CORES)))
    w = np.concatenate([r["out_w"] for r in res.results], axis=0)
    i = np.concatenate([r["out_i"] for r in res.results], axis=0)
    return w.astype(np.float32), i.astype(np.int32)
